# revision 34
# baseline (speedup 1.0000x reference)
"""Trainium2 Bass kernel for the Backflow module.

Math (B=16, N=512, DIM=3, H=32):
  out[b,i,:] = sum_j eta(||x_bi - x_bj||) * (x_bi - x_bj)  +  mu(||x_bi||) * x_bi
where eta/mu are 1->H->1 tanh MLPs. The reference's eye()/diagonal correction
cancels exactly (eta(d_ii) multiplies r_ii = 0).

Key trick: eta and mu are smooth univariate functions, so at runtime the host
refits them as tiny tanh networks *in squared-distance space*:
  eta(d) ~ sum_{k<3} w2_k tanh(w1_k u + b1_k) + b2,   u = d^2
  mu(r)  ~ sum_{k<5} v2_k tanh(v1_k n + c1_k) + c2,   n = r^2
(weighted least squares on a grid; measured end-to-end error vs the 32-unit
truth is ~8e-4, far below the 2e-2 gate). Fitting in u-space removes every
Sqrt from the device program: one ACT table set serves the whole kernel, and
tanh can read the d^2 values as soon as they land. The w2 are returned
fp16-exact via a greedy quantize-and-resolve pass (large canceling w2 pairs
are poison: their fp16 rounding error is amplified ~50x by the row sums).

Per-core (2 batches/core on 8 cores, all small tensors replicated):
  d^2 strips on the PE in f32r: stationary [-2x_i | 1 | n_i] x moving
  [x_j | n_j | 1] -> PSUM = d^2 directly (no clamp: tanh, unlike sqrt, is
  happy with the slightly-negative diagonal f32r leaves). DVE stages u in
  SBUF fp16. i on partitions (4 chunks of 128), j on the free dim,
  block-triangular strips packed to [128, 1280] (symmetry: chunk I covers
  j >= 128*I; reflected blocks come from PE transposes of the fp16 G strip).

  G accumulated in PSUM by 3 tanh ACT passes x diag(w2_k) fp16 matmuls
  (k0 split 512+808 to start one cast earlier; mu's 5 units ride batch 0's
  k0 pass as extra pre-inverted-affine columns, combined by tiny DVE ops).

  Contractions use the G blocks as PE *stationary* with the 6-wide moving
  [x_c | 1]: one matmul per 128x128 block -> per-row-chunk [128, 6] PSUM
  (P in cols 0:3, Q in 3:6, i-major so no partition-offset reads).
  Adjacent row chunks alternate between two PSUM tiles and only the first
  matmul of a tile uses start=True (a later bank reset would WAR-serialize
  against the previous row's finalize reads). b2*S_c lands in PSUM via an
  ident-stationary matmul. Finalize per row is 2 DVE ops:
    out[i,c] = x[i,c]*(Q_i + mu_i + mu_b2 + b2*N) - (P[i,c] + b2*S_c)
  written [P, NCHUNK, DIM] and DMA'd in partition-major layout (one
  contiguous 48B run per partition; the host reshapes).

Latency engineering: 6 dummy PE matmuls at program start ramp the PE clock
governor (0.65 -> 2.4 GHz) inside the input-DMA window; input DMAs spread
across the gpsimd/sync/scalar queues; each batch's reflection work drains
inside the next batch's k-loop; the last batch's acc copies use the
freshly-idle ACT engine.
"""

import sys

sys.path.insert(0, "/opt/trn_rl_repo")

import numpy as np
from contextlib import ExitStack

B, N, DIM, H = 16, 512, 3, 32
HP = 3  # refitted eta units (u-space)
HM = 5  # refitted mu units (n-space)
NMU = HM * 2 * 4  # mu-arg columns appended to batch 0 ds (HM units x BPC x NCHUNK)
NCORES = 8
BPC = B // NCORES  # batches per core
P = 128
NCHUNK = N // P  # 4
NROW = DIM + 2  # d^2 matmul contraction rows: x(3), n_j, ones
# block-triangular strips: chunk I covers j in [128*I, N)
WIDTHS = [N - P * I for I in range(NCHUNK)]  # [512, 384, 256, 128]
OFFS = [0]
for w in WIDTHS[:-1]:
    OFFS.append(OFFS[-1] + w)
NPACK = sum(WIDTHS)  # 1280
# PSUM-bank-sized column splits of the packed strip for the diag matmuls
MM_SPLITS = [(0, 512), (512, 512), (1024, 256)]
N_WARMUP = 6  # dummy PE matmuls to ramp the clock governor

LAST_RESULT = None
_PROGRAM_CACHE = {}


def _spread_sync_waits(nc):
    """The pinned walrus rejects instructions carrying more than one sync wait
    ('Too many sync wait commands'). Engines execute their instruction streams
    in order, so hoist all-but-one wait of any such instruction onto same-engine
    NoOps inserted directly before it — semantically identical ordering."""
    from concourse import mybir

    n_added = 0
    for bb in nc.main_func.blocks:
        insts = bb.instructions
        i = 0
        while i < len(insts):
            inst = insts[i]
            si = getattr(inst, "sync_info", None)
            waits = list(si.on_wait) if si is not None and si.on_wait else []
            if len(waits) > 1:
                si.on_wait = waits[-1:]
                for k, w in enumerate(waits[:-1]):
                    nop = mybir.InstNoOp(
                        name=f"{inst.name}-wspread{k}",
                        sync_info=mybir.SyncInfo(on_wait=[w], on_update=[]),
                        engine=inst.engine,
                        bass_nofuse=True,
                    )
                    insts.insert(i + k, nop)
                    n_added += 1
                i += len(waits) - 1
            i += 1
    return n_added


def _build_program():
    import concourse.bass as bass
    import concourse.tile as tile
    from concourse import mybir

    f32 = mybir.dt.float32
    f32r = mybir.dt.float32r
    f16 = mybir.dt.float16
    AF = mybir.ActivationFunctionType
    OP = mybir.AluOpType

    nc = bass.Bass()
    xTn_d = nc.dram_tensor("xTn", [NROW, BPC, N], f32r, kind="ExternalInput")
    statd_d = nc.dram_tensor("statd", [NROW, BPC, NCHUNK, P], f32r, kind="ExternalInput")
    xin2_d = nc.dram_tensor("xin2", [P, BPC, NCHUNK], f32, kind="ExternalInput")
    statx_d = nc.dram_tensor("statx", [P, BPC, NCHUNK, 2 * DIM], f16, kind="ExternalInput")
    etas_d = nc.dram_tensor("etas", [P, 2, HP], f32, kind="ExternalInput")
    w2d_d = nc.dram_tensor("w2d", [P, HP, P], f16, kind="ExternalInput")
    ident_d = nc.dram_tensor("ident", [P, P], f16, kind="ExternalInput")
    bs6_d = nc.dram_tensor("bs6", [P, BPC, 2 * DIM], f16, kind="ExternalInput")
    mupre_d = nc.dram_tensor("mupre", [P, 2, HM], f32, kind="ExternalInput")
    muc_d = nc.dram_tensor("muc", [P, HM + 1], f32, kind="ExternalInput")
    out_d = nc.dram_tensor("out", [BPC, P, NCHUNK, DIM], f32, kind="ExternalOutput")

    with tile.TileContext(nc) as tc, ExitStack() as ctx:
        singles = ctx.enter_context(tc.tile_pool(name="singles", bufs=1))
        dqp = ctx.enter_context(tc.tile_pool(name="dqp", bufs=1))
        hp0 = ctx.enter_context(tc.tile_pool(name="hp0", bufs=3))
        accsbp = ctx.enter_context(tc.tile_pool(name="accsbp", bufs=2))
        atp = ctx.enter_context(tc.tile_pool(name="atp", bufs=8))
        enp = ctx.enter_context(tc.tile_pool(name="enp", bufs=2))
        orp = ctx.enter_context(tc.tile_pool(name="orp", bufs=2))
        mup = ctx.enter_context(tc.tile_pool(name="mup", bufs=2))
        psacc = ctx.enter_context(tc.tile_pool(name="psacc", bufs=1, space="PSUM"))
        psout = ctx.enter_context(tc.tile_pool(name="psout", bufs=1, space="PSUM"))
        psd2 = ctx.enter_context(tc.tile_pool(name="psd2", bufs=3, space="PSUM"))

        # ---- PE warmup: data-independent matmuls ramp the clock governor
        # during the input-DMA window (cold PE runs at 0.65 GHz, warm 2.4).
        wu_sb = singles.tile([P, 512], f16)
        nc.gpsimd.memset(wu_sb[:], 0.25)
        for _ in range(N_WARMUP):
            wt = psd2.tile([P, 512], f32, tag="d2")
            nc.tensor.matmul(wt[:], wu_sb[:, 0:P], wu_sb[:], start=True, stop=True)

        # ---- input DMAs, spread across four engine queues so the per-issue
        # overhead (~0.7us each) runs in parallel; d^2-path tensors first.
        xTn_sb = singles.tile([NROW, BPC, N], f32r)
        nc.gpsimd.dma_start(out=xTn_sb[:], in_=xTn_d[:])
        statd_sb = singles.tile([NROW, BPC, NCHUNK, P], f32r)
        nc.sync.dma_start(out=statd_sb[:], in_=statd_d[:])
        etas_sb = singles.tile([P, 2, HP], f32)
        nc.scalar.dma_start(out=etas_sb[:], in_=etas_d[:])
        w2d_sb = singles.tile([P, HP, P], f16)
        nc.scalar.dma_start(out=w2d_sb[:], in_=w2d_d[:])
        statx_sb = singles.tile([P, BPC, NCHUNK, 2 * DIM], f16)
        nc.scalar.dma_start(out=statx_sb[:], in_=statx_d[:])
        ident_sb = singles.tile([P, P], f16)
        nc.scalar.dma_start(out=ident_sb[:], in_=ident_d[:])
        xin2_sb = singles.tile([P, BPC, NCHUNK], f32)
        nc.sync.dma_start(out=xin2_sb[:], in_=xin2_d[:])
        bs6_sb = singles.tile([P, BPC, 2 * DIM], f16)
        nc.sync.dma_start(out=bs6_sb[:], in_=bs6_d[:])
        mupre_sb = singles.tile([P, 2, HM], f32)
        nc.sync.dma_start(out=mupre_sb[:], in_=mupre_d[:])
        muc_sb = singles.tile([P, HM + 1], f32)
        nc.sync.dma_start(out=muc_sb[:], in_=muc_d[:])

        # ---- mu rides batch 0's k0 eta-tanh pass: columns 1280+ of ds hold
        # pre-inverted affine args (v1_j*n + c1_j - b1_0)/w1_0 so that
        # tanh(w1_0 * col + b1_0) = tanh(v1_j*n + c1_j); the m combination
        # happens on DVE right after k0.
        mu_box = {}

        # ---- per-(batch, chunk) d^2 matmul + fp16 staging copy ----
        ds_all = {
            b: dqp.tile(
                [P, NPACK + (NMU if b == 0 else 0)],
                f16,
                tag=f"ds{b}",
                name=f"ds{b}",
            )
            for b in range(BPC)
        }

        def prep(b, I):
            ds = ds_all[b]
            d2ps = psd2.tile([P, WIDTHS[I]], f32, tag="d2")
            nc.tensor.matmul(
                d2ps[:],
                statd_sb[:, b, I, :],
                xTn_sb[:, b, P * I : N],
                start=True,
                stop=True,
            )
            sl = ds[:, OFFS[I] : OFFS[I] + WIDTHS[I]]
            if I == 0 and b == 0:
                nc.scalar.copy(sl, d2ps[:])
            else:
                nc.vector.tensor_copy(sl, d2ps[:])

        def make_reflection(b, acc_sb):
            """Return closures for the contractions (G blocks as PE
            stationary, [x|1] 6-wide moving -> per-row [128, 6] PSUM with P
            in cols 0:3, Q in 3:6), transposes for the reflected blocks, and
            per-row finalize + per-row output DMA."""
            def blk(I, J):
                off = OFFS[I] + (J - I) * P
                return acc_sb[:, off : off + P]

            # adjacent rows go to different PSUM tiles (banks) so a row's
            # bank reset / accumulation never WAR-serializes against the
            # previous row's finalize reads
            pqa = psout.tile([P, 2, 2 * DIM], f32, tag="pqa")
            pqb = psout.tile([P, 2, 2 * DIM], f32, tag="pqb")

            def pq_slot(row):
                return (pqa, pqb)[row % 2], row // 2

            nfirst = {id(pqa): True, id(pqb): True}

            def contrib(row, stat_chunk, stationary, last=False):
                t, r = pq_slot(row)
                nc.tensor.matmul(
                    t[:, r, :],
                    stationary,
                    statx_sb[:, b, stat_chunk, :],
                    start=nfirst[id(t)],
                    stop=last,
                    skip_group_check=True,
                )
                nfirst[id(t)] = False

            def bs_add(row):
                # += [b2*S_c | 0] via ident-stationary matmul (folds the
                # bias-row correction into PSUM, shortening the fin chain)
                t, r = pq_slot(row)
                nc.tensor.matmul(
                    t[:, r, :],
                    ident_sb[:],
                    bs6_sb[:, b, :],
                    start=False,
                    stop=True,
                    skip_group_check=True,
                )

            at_tiles = {}

            def trans_only(I, J):
                tps = psd2.tile([P, P], f16, tag="d2")
                nc.tensor.transpose(tps[:], blk(I, J), ident_sb[:])
                at_sb = atp.tile([P, P], f16)
                nc.vector.tensor_copy(at_sb[:], tps[:])
                at_tiles[(I, J)] = at_sb

            outrow = orp.tile([P, NCHUNK, DIM], f32)

            def fin_row(R):
                # out[i,c] = x[i,c]*(Q_i + m_i) - (P[i,c] + b2*S_c)
                pt, r = pq_slot(R)
                t = enp.tile([P, DIM], f32, tag="t")
                nc.vector.scalar_tensor_tensor(
                    out=t[:], in0=pt[:, r, DIM : 2 * DIM],
                    scalar=mu_box["m"][:, b, R : R + 1],
                    in1=statx_sb[:, b, R, 0:DIM],
                    op0=OP.add, op1=OP.mult,
                )
                nc.vector.tensor_sub(outrow[:, R, :], t[:], pt[:, r, 0:DIM])

            # row 3 (transpose-free) first, then all transposes densely,
            # then rows 2..0
            ops = []
            row = NCHUNK - 1
            ops.append(lambda row=row: contrib(row, row, blk(row, row)))
            for I in range(row):
                ops.append(lambda row=row, I=I: contrib(row, I, blk(I, row)))
            ops.append(lambda row=row: bs_add(row))
            ops.append(lambda row=row: fin_row(row))
            for I in range(NCHUNK):
                for J in range(I + 1, NCHUNK):
                    ops.append(lambda I=I, J=J: trans_only(I, J))
            for row in range(NCHUNK - 2, -1, -1):
                ops.append(lambda row=row: contrib(row, row, blk(row, row)))
                for I in range(row):
                    ops.append(lambda row=row, I=I: contrib(row, I, blk(I, row)))
                for J in range(row + 1, NCHUNK):
                    ops.append(
                        lambda row=row, J=J: contrib(row, J, at_tiles[(row, J)][:])
                    )
                ops.append(lambda row=row: bs_add(row))
                ops.append(lambda row=row: fin_row(row))
            ops.append(lambda: nc.sync.dma_start(out=out_d[b], in_=outrow[:]))
            return ops

        # ---- main per-batch flow ----
        en_all = {}
        pending = []
        for b in range(BPC):
            for I in range(NCHUNK):
                prep(b, I)
        for b in range(BPC):
            ds = ds_all[b]
            if b == 0:
                # mu-arg columns (both batches' n_i) written by DVE
                for j in range(HM):
                    nc.vector.tensor_scalar(
                        out=ds[:, NPACK + 8 * j : NPACK + 8 * (j + 1)].rearrange(
                            "p (u v) -> p u v", u=BPC
                        ),
                        in0=xin2_sb[:],
                        scalar1=mupre_sb[:, 0, j : j + 1],
                        scalar2=mupre_sb[:, 1, j : j + 1],
                        op0=OP.mult,
                        op1=OP.add,
                    )
            acc = psacc.tile([P, NPACK], f32)
            acc_sb = accsbp.tile([P, NPACK], f16)
            for k in range(HP):
                hs = hp0.tile([P, NPACK + NMU], f16, tag="hs")
                if k == 0:
                    # split: first 512 columns right after cast c0, the rest
                    # (plus mu columns for batch 0) once all casts land
                    nc.scalar.activation(
                        hs[:, 0:512],
                        ds[:, 0:512],
                        AF.Tanh,
                        scale=etas_sb[:, 0, 0:1],
                        bias=etas_sb[:, 1, 0:1],
                    )
                    hi = NPACK + (NMU if b == 0 else 0)
                    nc.scalar.activation(
                        hs[:, 512:hi],
                        ds[:, 512:hi],
                        AF.Tanh,
                        scale=etas_sb[:, 0, 0:1],
                        bias=etas_sb[:, 1, 0:1],
                    )
                else:
                    nc.scalar.activation(
                        hs[:, 0:NPACK],
                        ds[:, 0:NPACK],
                        AF.Tanh,
                        scale=etas_sb[:, 0, k : k + 1],
                        bias=etas_sb[:, 1, k : k + 1],
                    )
                for off, w in MM_SPLITS:
                    nc.tensor.matmul(
                        acc[:, off : off + w],
                        w2d_sb[:, k, :],
                        hs[:, off : off + w],
                        start=(k == 0),
                        stop=(k == HP - 1),
                    )
                if b == 0 and k == 0:
                    # combine mu units: m = sum_j v2_j tanh_j + (c2 + b2*N)
                    for j in range(HM):
                        sl = hs[:, NPACK + 8 * j : NPACK + 8 * (j + 1)].rearrange(
                            "p (u v) -> p u v", u=BPC
                        )
                        mnew = mup.tile([P, BPC, NCHUNK], f32, tag="mu", bufs=2)
                        if j == 0:
                            nc.vector.scalar_tensor_tensor(
                                out=mnew[:],
                                in0=sl,
                                scalar=muc_sb[:, 0:1],
                                in1=muc_sb[:, HM : HM + 1].to_broadcast(
                                    [P, BPC, NCHUNK]
                                ),
                                op0=OP.mult,
                                op1=OP.add,
                            )
                        else:
                            nc.vector.scalar_tensor_tensor(
                                out=mnew[:],
                                in0=sl,
                                scalar=muc_sb[:, j : j + 1],
                                in1=mu_box["m"][:],
                                op0=OP.mult,
                                op1=OP.add,
                            )
                        mu_box["m"] = mnew
                if k == HP - 1:
                    # stage acc to SBUF fp16; the last batch uses the
                    # freshly-idle ACT for two of the three copies
                    for off, w in MM_SPLITS:
                        if b == BPC - 1 and off != 512:
                            nc.scalar.copy(
                                acc_sb[:, off : off + w], acc[:, off : off + w]
                            )
                        else:
                            nc.vector.tensor_copy(
                                acc_sb[:, off : off + w], acc[:, off : off + w]
                            )
                # drain previous batch's reflection work into this k-loop
                for _ in range(8):
                    if pending:
                        pending.pop(0)()
            while pending:
                pending.pop(0)()
            pending = make_reflection(b, acc_sb)
        while pending:
            pending.pop(0)()

    _spread_sync_waits(nc)
    return nc


def _fit_tanh(tgrid, target, wts, hp, seed=1, n_starts=14, tol=3e-5):
    """Weighted LS refit of a smooth 1-D function as hp tanh units.

    The w2 coefficients end up as an fp16 diag-matmul stationary on device.
    A strong ridge (lam=5e-2) keeps |w2| small — large *canceling* w2 pairs
    are poison: their fp16 rounding breaks the cancellation and the 512-term
    row sums amplify the systematic eta error ~50x. A final greedy
    quantization pass rounds w2 to fp16 one coefficient at a time (largest
    first), re-solving the remaining coefficients + bias against the
    residual, so the returned w2 are exactly representable in fp16."""
    from scipy.optimize import least_squares

    def lin_solve(w1, b1, lam=5e-2):
        Phi = np.concatenate([np.tanh(tgrid[:, None] * w1 + b1), np.ones((len(tgrid), 1))], 1)
        A = Phi * wts[:, None]
        Aaug = np.concatenate([A, lam * np.eye(Phi.shape[1])], axis=0)
        baug = np.concatenate([target * wts, np.zeros(Phi.shape[1])])
        cc, *_ = np.linalg.lstsq(Aaug, baug, rcond=None)
        return cc, Phi

    rng = np.random.default_rng(seed)
    tmax = max(tgrid.max(), 1.0)
    best = None
    for _ in range(n_starts):
        w1 = rng.uniform(0.1, 1.0, hp) * rng.choice([-1, 1], hp) / tmax
        b1 = -w1 * rng.uniform(tgrid.min(), tmax, hp)

        def resid(p):
            cc, Phi = lin_solve(p[:hp], p[hp:])
            return (Phi @ cc - target) * wts

        try:
            res = least_squares(
                resid, np.concatenate([w1, b1]), method="lm", max_nfev=300
            )
        except Exception:
            continue
        if best is None or res.cost < best[0]:
            best = (res.cost, res.x)
        if best[0] < tol:
            break
    _, p = best
    w1o, b1o = p[:hp], p[hp:]
    cc, _ = lin_solve(w1o, b1o)
    w2o = cc[:hp].copy()

    # greedy fp16-aware rounding of w2 (ridge-stabilized re-solves)
    Phi = np.tanh(tgrid[:, None] * w1o + b1o)
    A = Phi * wts[:, None]
    y = target * wts
    w2q = np.zeros(hp)
    b2o = float(cc[hp])
    free = list(range(hp))
    for _ in range(hp):
        j = max(free, key=lambda i: abs(w2o[i]))
        w2q[j] = float(np.float16(w2o[j]))
        free.remove(j)
        fixed = [i for i in range(hp) if i not in free]
        y2 = y - A[:, fixed] @ w2q[fixed]
        Af = np.concatenate([A[:, free], wts[:, None]], 1)
        nf = Af.shape[1]
        Aaug = np.concatenate([Af, 1e-3 * np.eye(nf)], 0)
        yaug = np.concatenate([y2, np.zeros(nf)])
        cc2, *_ = np.linalg.lstsq(Aaug, yaug, rcond=None)
        if free:
            w2o[free] = cc2[:-1]
        b2o = float(cc2[-1])
    return w1o, b1o, w2q, b2o


def _ensure_ntff_hook():
    """bass_utils' axon trace path imports antenv.axon_hooks, which the image's
    antenv package lacks. Register an equivalent module backed by the boot
    package's ctypes NTFF hook so trace=True works; degrade silently if the
    pieces are missing (tracing is optional)."""
    import os
    import types

    try:
        import antenv.axon_hooks  # noqa: F401

        return
    except ImportError:
        pass
    try:
        import antenv
    except ImportError:
        return
    mod = types.ModuleType("antenv.axon_hooks")
    box = {"h": None}
    mod.set_axon_ntff_profile_hook = lambda h: box.__setitem__("h", h)
    mod.get_axon_ntff_profile_hook = lambda: box["h"]
    sys.modules["antenv.axon_hooks"] = mod
    antenv.axon_hooks = mod
    try:
        from trn_agent_boot.trn_boot import _ntff_profile_via_ctypes

        so = "/opt/axon/libaxon_pjrt.so"
        if os.path.exists(so):
            hook = _ntff_profile_via_ctypes(so)
            if hook is not None:
                mod.set_axon_ntff_profile_hook(hook)
    except Exception:
        pass


def kernel(x, eta_w1, eta_b1, eta_w2, eta_b2, mu_w1, mu_b1, mu_w2, mu_b2):
    global LAST_RESULT
    _ensure_ntff_hook()
    from concourse.bass_utils import run_bass_kernel_spmd

    f32 = np.float32
    f16 = np.float16
    x = np.ascontiguousarray(np.asarray(x, dtype=f32))
    eta_w1 = np.asarray(eta_w1, f32)
    eta_b1 = np.asarray(eta_b1, f32)
    eta_w2 = np.asarray(eta_w2, f32)
    eta_b2 = np.asarray(eta_b2, f32)
    mu_w1 = np.asarray(mu_w1, f32)
    mu_b1 = np.asarray(mu_b1, f32)
    mu_w2 = np.asarray(mu_w2, f32)
    mu_b2 = np.asarray(mu_b2, f32)

    norms = np.linalg.norm(x, axis=2)
    dmax = 2.0 * norms.max()
    # eta refit in u = d^2 space
    dg = np.linspace(0.0, dmax, 1200)
    eta_t = np.tanh(dg[:, None] * eta_w1[0].astype(np.float64) + eta_b1) @ eta_w2[
        :, 0
    ].astype(np.float64) + float(eta_b2[0])
    w1f, b1f, w2f, b2f = _fit_tanh(dg * dg, eta_t, np.maximum(dg, 0.05), HP)
    # mu refit in n = r^2 space
    rg = np.linspace(0.0, norms.max() * 1.02, 800)
    mu_t = np.tanh(rg[:, None] * mu_w1[0].astype(np.float64) + mu_b1) @ mu_w2[
        :, 0
    ].astype(np.float64) + float(mu_b2[0])
    v1f, c1f, v2f, c2f = _fit_tanh(rg * rg, mu_t, np.ones_like(rg), HM)

    if "prog" not in _PROGRAM_CACHE:
        _PROGRAM_CACHE["prog"] = _build_program()
    nc = _PROGRAM_CACHE["prog"]

    w2d = np.zeros((P, HP, P), f16)
    idx = np.arange(P)
    w2d[idx, :, idx] = w2f.astype(f16)[None, :]
    etas = np.zeros((P, 2, HP), f32)
    etas[:, 0, :] = w1f[None, :]
    etas[:, 1, :] = b1f[None, :]
    ident = np.eye(P, dtype=f16)
    mupre = np.zeros((P, 2, HM), f32)
    mupre[:, 0, :] = (v1f / w1f[0])[None, :]
    mupre[:, 1, :] = ((c1f - b1f[0]) / w1f[0])[None, :]
    muc = np.zeros((P, HM + 1), f32)
    muc[:, 0:HM] = v2f[None, :]
    muc[:, HM] = c2f + b2f * N

    in_maps = []
    for core in range(NCORES):
        xc = np.ascontiguousarray(x[core * BPC : (core + 1) * BPC])
        xTc = xc.transpose(0, 2, 1)  # [BPC, DIM, N]
        n2 = (xc**2).sum(axis=2)  # [BPC, N]
        xTn = np.empty((NROW, BPC, N), f32)
        xTn[0:DIM] = xTc.transpose(1, 0, 2)
        xTn[DIM] = n2
        xTn[DIM + 1] = 1.0
        statd = np.empty((NROW, BPC, NCHUNK, P), f32)
        statx = np.empty((P, BPC, NCHUNK, 2 * DIM), f16)
        xin2 = np.empty((P, BPC, NCHUNK), f32)
        for bb in range(BPC):
            for I in range(NCHUNK):
                statd[0:DIM, bb, I, :] = -2.0 * xTc[bb, :, I * P : (I + 1) * P]
                statd[DIM, bb, I, :] = 1.0
                statd[DIM + 1, bb, I, :] = n2[bb, I * P : (I + 1) * P]
                statx[:, bb, I, 0:DIM] = xc[bb, I * P : (I + 1) * P].astype(f16)
                statx[:, bb, I, DIM : 2 * DIM] = 1.0
                xin2[:, bb, I] = n2[bb, I * P : (I + 1) * P]
        S = xc.sum(axis=1)  # [BPC, DIM]
        bs6 = np.zeros((P, BPC, 2 * DIM), f16)
        bs6[:, :, 0:DIM] = (b2f * S).astype(f16)[None]
        in_maps.append(
            {
                "xTn": np.ascontiguousarray(xTn),
                "statd": statd,
                "xin2": xin2,
                "statx": statx,
                "etas": etas,
                "w2d": w2d,
                "ident": ident,
                "bs6": bs6,
                "mupre": mupre,
                "muc": muc,
            }
        )

    res = run_bass_kernel_spmd(nc, in_maps, core_ids=list(range(NCORES)))
    LAST_RESULT = res
    out = np.concatenate([r["out"] for r in res.results], axis=0)  # [B, P, NCHUNK, DIM]
    out = out.transpose(0, 2, 1, 3).reshape(B, N, DIM)
    return np.ascontiguousarray(out).astype(np.float32)
